# revision 5
# baseline (speedup 1.0000x reference)
"""Trainium2 Bass kernel for MultiHeadNodeToEdgeAttention (hypergraph node->edge).

Contract: kernel(**inputs) takes FULL unsharded inputs (numpy), returns the FULL
[E, OUT_DIM] float32 output.  Internally shards the incidence matrix along the
hyperedge axis E across 8 NeuronCores; node features and (folded) head weights
are replicated.  Softmax-over-E and min-max-normalization globals are resolved
with a single 3 KB AllGather of per-core stats.

Key algebraic folds (exact, done on host in float64):
  m[h]  = inc^T @ (nf @ W1[h])          = (inc^T @ nf) @ W1[h]
      ->  g = nf^T @ inc computed ONCE (head-independent), per-head work folds
          into 128x128 / 128x4 weight matrices applied to g.
  scores[h] = m[h] @ Wa[h] + ba[h]      -> (W1[h] @ Wa[h]) applied to g
  u~[h] = m[h] @ W2[h]                  -> (W1[h] @ W2[h]) applied to g
  b2 cancels exactly inside min-max normalization:
  (u - mn)/(mx - mn + eps) == (v - vmin)/(vmax - vmin + Z*eps)
  where v = exp(s - smax) * u~,  u = v/Z + b2.
"""

import os

import numpy as np

import bass_rust
import concourse.bass as bass
import concourse.mybir as mybir
import concourse.tile as tile
from concourse import bass_utils
from concourse.vector_clock import ScopedClock

# ---------------------------------------------------------------- constants
N_CORES = 8
NODE_DIM, EDGE_DIM, HIDDEN, OUT_DIM, HEADS = 128, 64, 128, 64, 4
N_NODES, N_EDGES = 4096, 16384
EPS = 1e-8
E_S = N_EDGES // N_CORES          # 2048 edges per core
NCH = N_NODES // 128              # 32 node chunks
ECH = 512                         # matmul moving-dim chunk
NEC = E_S // ECH                  # 4 e-chunks
NSTACK = 2                        # head pairs stacked on 128 partitions

F32 = mybir.dt.float32
_MM_DT_NAME = os.environ.get("BASS_MM_DT", "f32r")
_MM_DT = {
    "f32": mybir.dt.float32,
    "f32r": mybir.dt.float32r,
    "f16": mybir.dt.float16,
    "bf16": mybir.dt.bfloat16,
}[_MM_DT_NAME]
_MM_NP = {"f32": np.float32, "f32r": np.float32,
          "f16": np.float16, "bf16": None}[_MM_DT_NAME]

# ------------------------------------------------- walrus single-wait fixes
# The pinned walrus build accepts at most ONE semaphore wait per instruction.
# Tile attaches several to the final drain and to ordinary instructions, so:
#  1) the drain keeps its waits (split afterwards like everything else),
#  2) after tracing, split every instruction with >1 waits into preceding
#     same-engine no-op carriers holding one wait each.


def _patched_drain_and_barrier(self, tick_clock, wait_clock):
    drain_inst = self.nc.sync.drain()
    wait_clock.add_sem_waits(
        drain_inst.ins, ScopedClock({None: tick_clock.global_clock})
    )
    self.nc.all_engine_barrier()
    assert self.sems is not None
    popped = self.nc._tile_sem_poison_stack.pop()
    assert popped is self._sem_poison
    self.nc.clear_and_free_semaphores(list(self.sems.allocated().values()))
    self.nc.all_engine_barrier()


tile.TileContext._drain_and_barrier = _patched_drain_and_barrier


def _split_excess_waits(nc, maxw=1):
    for f in nc.m.functions:
        for bb in f.blocks:
            out = []
            changed = False
            for inst in bb.instructions:
                si = inst.sync_info
                waits = list(si.on_wait) if si is not None else []
                if len(waits) > maxw:
                    changed = True
                    extra, keep = waits[:-maxw], waits[-maxw:]
                    for i in range(0, len(extra), maxw):
                        nop = nc.engines[inst.engine].nop(nofuse=True)
                        ni = nop.ins
                        cb = nc.cur_bb.bb
                        assert cb.instructions[-1].name == ni.name
                        cb.instructions = cb.instructions[:-1]
                        ni.sync_info = bass_rust.SyncInfo(
                            on_wait=extra[i:i + maxw], on_update=[]
                        )
                        out.append(ni)
                    inst.sync_info = bass_rust.SyncInfo(
                        on_wait=keep, on_update=list(si.on_update)
                    )
                out.append(inst)
            if changed:
                bb.instructions = out


# ---------------------------------------------------------------- bass trace
def _build_nc():
    nc = bass.Bass("TRN2", target_bir_lowering=False, debug=False,
                   num_devices=N_CORES)

    inc = nc.dram_tensor("inc", [N_NODES, E_S], _MM_DT, kind="ExternalInput").ap()
    nf = nc.dram_tensor("nf", [N_NODES, 128], _MM_DT, kind="ExternalInput").ap()
    w2e = nc.dram_tensor("w2e", [NSTACK, 128, 128], F32, kind="ExternalInput").ap()
    wa = nc.dram_tensor("wa", [128, HEADS], F32, kind="ExternalInput").ap()
    ba = nc.dram_tensor("ba", [HEADS, 1], F32, kind="ExternalInput").ap()
    sel = nc.dram_tensor("sel", [NSTACK, HEADS, 128], F32, kind="ExternalInput").ap()
    wout = nc.dram_tensor("wout", [NSTACK, 128, OUT_DIM], F32, kind="ExternalInput").ap()
    bout = nc.dram_tensor("bout", [OUT_DIM, 1], F32, kind="ExternalInput").ap()
    bias_u = nc.dram_tensor("bias_u", [NSTACK, 128, 1], F32, kind="ExternalInput").ap()
    out_T = nc.dram_tensor("out_T", [OUT_DIM, E_S], F32, kind="ExternalOutput").ap()

    inc_r = inc.rearrange("(c p) e -> c p e", p=128)       # [32, 128, 2048]
    nf_r = nf.rearrange("(c p) d -> p c d", p=128)         # [128, 32, 128]

    Exp = mybir.ActivationFunctionType.Exp
    Relu = mybir.ActivationFunctionType.Relu
    Ident = mybir.ActivationFunctionType.Identity
    AX = mybir.AxisListType.X
    AXY = mybir.AxisListType.XY
    MUL = mybir.AluOpType.mult
    ADD = mybir.AluOpType.add
    MAX = mybir.AluOpType.max
    MIN = mybir.AluOpType.min

    with tile.TileContext(nc) as tc:
        with (
            tc.tile_pool(name="wpool", bufs=1) as wp,
            tc.tile_pool(name="incp", bufs=4) as incp,
            tc.tile_pool(name="big", bufs=1) as bg,
            tc.tile_pool(name="small", bufs=1) as sm,
            tc.tile_pool(name="pg", bufs=4, space="PSUM") as pg,
            tc.tile_pool(name="psc", bufs=1, space="PSUM") as psc,
            tc.tile_pool(name="ppb", bufs=2, space="PSUM") as ppb,
            tc.tile_pool(name="pu", bufs=1, space="PSUM") as pu,
            tc.tile_pool(name="dram", bufs=1, space="DRAM") as dram,
        ):
            # ---- resident weights / node features
            nf_t = wp.tile([128, NCH, 128], _MM_DT)
            nc.sync.dma_start(nf_t[:], nf_r[:])
            wa_t = wp.tile([128, HEADS], F32)
            nc.sync.dma_start(wa_t[:], wa[:])
            ba_t = wp.tile([HEADS, 1], F32)
            nc.sync.dma_start(ba_t[:], ba[:])
            sel_t = wp.tile([HEADS, NSTACK, 128], F32)
            nc.sync.dma_start(sel_t[:], sel.rearrange("s h p -> h s p"))
            bout_t = wp.tile([OUT_DIM, 1], F32)
            nc.sync.dma_start(bout_t[:], bout[:])
            w2e_t2 = wp.tile([128, NSTACK, 128], F32)
            nc.sync.dma_start(w2e_t2[:], w2e.rearrange("s d k -> d s k"))
            wout_t2 = wp.tile([128, NSTACK, OUT_DIM], F32)
            nc.sync.dma_start(wout_t2[:], wout.rearrange("s p o -> p s o"))
            bias_u2 = wp.tile([128, NSTACK], F32)
            nc.sync.dma_start(bias_u2[:], bias_u.rearrange("s p one -> p (s one)"))

            # ---- stage B: g_T[d, e] = sum_n nf[n, d] * inc[n, e]
            g_ps = [pg.tile([128, ECH], F32, tag="g", name=f"g{i}") for i in range(NEC)]
            for c in range(NCH):
                inc_t = incp.tile([128, E_S], _MM_DT, tag="inc")
                nc.sync.dma_start(inc_t[:], inc_r[c])
                for ec in range(NEC):
                    nc.tensor.matmul(
                        g_ps[ec][:],
                        nf_t[:, c, :],
                        inc_t[:, ec * ECH:(ec + 1) * ECH],
                        start=(c == 0),
                        stop=(c == NCH - 1),
                    )

            # ---- stage C: per e-chunk epilogue off PSUM
            g_T = bg.tile([128, E_S], F32, tag="gT")
            s_raw = sm.tile([HEADS, E_S], F32, tag="sraw")
            for ec in range(NEC):
                sl = slice(ec * ECH, (ec + 1) * ECH)
                # g PSUM -> SBUF (ScalarE; fast PSUM read)
                nc.scalar.copy(g_T[:, sl], g_ps[ec][:])
                # scores chunk: [4, ech] = wa^T @ g
                sc_ps = psc.tile([HEADS, ECH], F32, tag="sc")
                nc.tensor.matmul(sc_ps[:], wa_t[:], g_T[:, sl],
                                 start=True, stop=True)
                nc.scalar.activation(s_raw[:, sl], sc_ps[:], Ident,
                                     bias=ba_t[:], scale=1.0)

            # leaky relu (slope .2): max(s, .2 s)
            s_lk = sm.tile([HEADS, E_S], F32, tag="slk")
            nc.vector.scalar_tensor_tensor(s_lk[:], s_raw[:], 0.2, s_raw[:],
                                           op0=MUL, op1=MAX)
            # local softmax pieces
            nsmax = sm.tile([HEADS, 1], F32, tag="nsmax")
            nc.vector.tensor_reduce(nsmax[:], s_lk[:], axis=AX, op=MAX,
                                    negate=True)
            smax_l = sm.tile([HEADS, 1], F32, tag="smaxl")
            nc.vector.tensor_scalar_mul(smax_l[:], nsmax[:], -1.0)
            p_sb = sm.tile([HEADS, E_S], F32, tag="psb")
            z_l = sm.tile([HEADS, 1], F32, tag="zl")
            nc.scalar.activation(p_sb[:], s_lk[:], Exp, bias=nsmax[:],
                                 scale=1.0, accum_out=z_l[:])

            # ---- u~ and v per stack
            v_sb = [bg.tile([128, E_S], F32, tag=f"v{s}", name=f"v{s}") for s in range(NSTACK)]
            u_sb = [bg.tile([128, E_S], F32, tag=f"u{s}", name=f"u{s}") for s in range(NSTACK)]
            for s in range(NSTACK):
                for ec in range(NEC):
                    sl = slice(ec * ECH, (ec + 1) * ECH)
                    u_ps = pu.tile([128, ECH], F32, tag="u")
                    nc.tensor.matmul(u_ps[:], w2e_t2[:, s, :], g_T[:, sl],
                                     start=True, stop=True)
                    nc.scalar.activation(u_sb[s][:, sl], u_ps[:], Ident,
                                         bias=bias_u2[:, s:s + 1], scale=1.0)
                    pb_ps = ppb.tile([128, ECH], F32, tag="pb")
                    nc.tensor.matmul(pb_ps[:], sel_t[:, s, :], p_sb[:, sl],
                                     start=True, stop=True)
                    nc.vector.tensor_tensor(v_sb[s][:, sl], u_sb[s][:, sl],
                                            pb_ps[:], op=MUL)

            # local per-(stack, partition) extrema over e
            vmin_l = [sm.tile([128, 1], F32, tag=f"vmin{s}", name=f"vmin{s}") for s in range(NSTACK)]
            vmax_l = [sm.tile([128, 1], F32, tag=f"vmax{s}", name=f"vmax{s}") for s in range(NSTACK)]
            for s in range(NSTACK):
                nc.vector.tensor_reduce(vmin_l[s][:], v_sb[s][:], axis=AX, op=MIN)
                nc.vector.tensor_reduce(vmax_l[s][:], v_sb[s][:], axis=AX, op=MAX)

            # ---- stats AllGather: [128, 6] per core -> [8, 128, 6]
            stats = sm.tile([128, 6], F32, tag="stats")
            nc.vector.memset(stats[:], 0.0)
            nc.vector.tensor_copy(stats[:, 0:1], vmin_l[0][:])
            nc.vector.tensor_copy(stats[:, 1:2], vmax_l[0][:])
            nc.vector.tensor_copy(stats[:, 2:3], vmin_l[1][:])
            nc.vector.tensor_copy(stats[:, 3:4], vmax_l[1][:])
            nc.vector.tensor_copy(stats[0:HEADS, 4:5], smax_l[:])
            nc.vector.tensor_copy(stats[0:HEADS, 5:6], z_l[:])

            cc_in = dram.tile([128, 6], F32)
            cc_out = dram.tile([N_CORES, 128, 6], F32, addr_space="Shared")
            nc.sync.dma_start(cc_in[:], stats[:])
            nc.gpsimd.collective_compute(
                "AllGather",
                mybir.AluOpType.bypass,
                ins=[cc_in[:]],
                outs=[cc_out[:]],
                replica_groups=[list(range(N_CORES))],
            )
            stats_all = sm.tile([128, 6, N_CORES], F32, tag="statsall")
            nc.sync.dma_start(stats_all[:], cc_out.rearrange("r p c -> p c r"))

            # ---- global reductions (tiny)
            neg_gsmax = sm.tile([HEADS, 1], F32, tag="ngsmax")
            nc.vector.tensor_reduce(neg_gsmax[:], stats_all[0:HEADS, 4, :],
                                    axis=AX, op=MAX, negate=True)
            c_all = sm.tile([HEADS, N_CORES], F32, tag="call")
            nc.scalar.activation(c_all[:], stats_all[0:HEADS, 4, :], Exp,
                                 bias=neg_gsmax[:], scale=1.0)
            zc = sm.tile([HEADS, N_CORES], F32, tag="zc")
            nc.vector.tensor_tensor(zc[:], stats_all[0:HEADS, 5, :], c_all[:],
                                    op=MUL)
            rhs2 = sm.tile([HEADS, 2], F32, tag="rhs2")
            nc.scalar.activation(rhs2[:, 0:1], smax_l[:], Exp,
                                 bias=neg_gsmax[:], scale=1.0)  # c_self
            nc.vector.tensor_reduce(rhs2[:, 1:2], zc[:], axis=AX, op=ADD)  # Z_g

            a_s = [sm.tile([128, 1], F32, tag=f"a{s}", name=f"a{s}") for s in range(NSTACK)]
            b_s = [sm.tile([128, 1], F32, tag=f"b{s}", name=f"b{s}") for s in range(NSTACK)]
            for s in range(NSTACK):
                cb_ps = ppb.tile([128, N_CORES], F32, tag="pb")
                nc.tensor.matmul(cb_ps[:], sel_t[:, s, :], c_all[:],
                                 start=True, stop=True)
                vminc = sm.tile([128, N_CORES], F32, tag="vminc")
                nc.vector.tensor_tensor(vminc[:], stats_all[:, 2 * s, :],
                                        cb_ps[:], op=MUL)
                vmaxc = sm.tile([128, N_CORES], F32, tag="vmaxc")
                nc.vector.tensor_tensor(vmaxc[:], stats_all[:, 2 * s + 1, :],
                                        cb_ps[:], op=MUL)
                vmin_g = sm.tile([128, 1], F32, tag="vming")
                nc.vector.tensor_reduce(vmin_g[:], vminc[:], axis=AX, op=MIN)
                vmax_g = sm.tile([128, 1], F32, tag="vmaxg")
                nc.vector.tensor_reduce(vmax_g[:], vmaxc[:], axis=AX, op=MAX)

                sel2_ps = pu.tile([128, 2], F32, tag="u")
                nc.tensor.matmul(sel2_ps[:], sel_t[:, s, :], rhs2[:],
                                 start=True, stop=True)
                diff = sm.tile([128, 1], F32, tag="diff")
                nc.vector.tensor_sub(diff[:], vmax_g[:], vmin_g[:])
                denom = sm.tile([128, 1], F32, tag="denom")
                nc.vector.scalar_tensor_tensor(denom[:], sel2_ps[:, 1:2], EPS,
                                               diff[:], op0=MUL, op1=ADD)
                rden = sm.tile([128, 1], F32, tag="rden")
                nc.vector.reciprocal(rden[:], denom[:])
                nc.vector.tensor_tensor(a_s[s][:], sel2_ps[:, 0:1], rden[:],
                                        op=MUL)
                nc.vector.scalar_tensor_tensor(b_s[s][:], vmin_g[:], -1.0,
                                               rden[:], op0=MUL, op1=MUL)

            # ---- normalize + relu + output matmul
            rv = [bg.tile([128, E_S], F32, tag=f"rv{s}", name=f"rv{s}") for s in range(NSTACK)]
            for s in range(NSTACK):
                nc.scalar.activation(rv[s][:], v_sb[s][:], Relu,
                                     bias=b_s[s][:], scale=a_s[s][:])
            out_sb = bg.tile([OUT_DIM, E_S], F32, tag="osb")
            for ec in range(NEC):
                sl = slice(ec * ECH, (ec + 1) * ECH)
                o_ps = psc.tile([OUT_DIM, ECH], F32, tag="sc")
                for s in range(NSTACK):
                    nc.tensor.matmul(o_ps[:], wout_t2[:, s, :], rv[s][:, sl],
                                     start=(s == 0), stop=(s == NSTACK - 1))
                nc.scalar.activation(out_sb[:, sl], o_ps[:], Ident,
                                     bias=bout_t[:], scale=1.0)
            nc.sync.dma_start(out_T[:], out_sb[:])

    _split_excess_waits(nc)
    return nc


_NC_CACHE = {}


def _get_nc():
    if "nc" not in _NC_CACHE:
        _NC_CACHE["nc"] = _build_nc()
    return _NC_CACHE["nc"]


# ------------------------------------------------------------- host wrapper
def _fold_weights(W1, b1, Wa, ba, W2, b2, Wout, bout):
    W1d = W1.astype(np.float64)
    b1d = b1.astype(np.float64)
    Wad = Wa.astype(np.float64)
    W2d = W2.astype(np.float64)

    wa_eff = np.einsum("hdk,hk->dh", W1d, Wad).astype(np.float32)      # [128,4]
    ba_eff = (ba.astype(np.float64)
              + np.einsum("hk,hk->h", b1d, Wad)).astype(np.float32)    # [4]
    W2eff = np.einsum("hdk,hko->hdo", W1d, W2d)                        # [4,128,64]
    biasu = np.einsum("hk,hko->ho", b1d, W2d)                          # [4,64]

    w2e = np.concatenate(
        [np.concatenate([W2eff[2 * s], W2eff[2 * s + 1]], axis=1)[None]
         for s in range(NSTACK)], axis=0).astype(np.float32)           # [2,128,128]
    bias_u = np.concatenate(
        [np.concatenate([biasu[2 * s], biasu[2 * s + 1]])[None]
         for s in range(NSTACK)], axis=0).astype(np.float32)[:, :, None]

    sel = np.zeros((NSTACK, HEADS, 128), np.float32)
    for s in range(NSTACK):
        sel[s, 2 * s, 0:64] = 1.0
        sel[s, 2 * s + 1, 64:128] = 1.0

    wout_s = np.stack([Wout[s * 128:(s + 1) * 128, :] for s in range(NSTACK)],
                      axis=0).astype(np.float32)                       # [2,128,64]
    return dict(
        w2e=w2e,
        wa=wa_eff,
        ba=ba_eff[:, None].astype(np.float32),
        sel=sel,
        wout=wout_s,
        bout=bout.astype(np.float32)[:, None],
        bias_u=bias_u,
    )


def kernel(node_features, incidence_matrix, W1, b1, Wa, ba, W2, b2, Wout, bout):
    node_features = np.asarray(node_features, np.float32)
    incidence_matrix = np.asarray(incidence_matrix, np.float32)
    weights = _fold_weights(np.asarray(W1), np.asarray(b1), np.asarray(Wa),
                            np.asarray(ba), np.asarray(W2), np.asarray(b2),
                            np.asarray(Wout), np.asarray(bout))

    if _MM_NP is np.float32:
        nf_in = node_features
        inc_full = incidence_matrix
    elif _MM_NP is None:  # bf16
        import ml_dtypes
        nf_in = node_features.astype(ml_dtypes.bfloat16)
        inc_full = incidence_matrix.astype(ml_dtypes.bfloat16)
    else:
        nf_in = node_features.astype(_MM_NP)
        inc_full = incidence_matrix.astype(_MM_NP)

    in_maps = []
    for c in range(N_CORES):
        shard = np.ascontiguousarray(inc_full[:, c * E_S:(c + 1) * E_S])
        in_maps.append({"inc": shard, "nf": nf_in, **weights})

    nc = _get_nc()
    res = bass_utils.run_bass_kernel_spmd(nc, in_maps,
                                          core_ids=list(range(N_CORES)))
    out = np.empty((N_EDGES, OUT_DIM), np.float32)
    for c in range(N_CORES):
        out[c * E_S:(c + 1) * E_S, :] = res.results[c]["out_T"].T
    return out
